# revision 33
# baseline (speedup 1.0000x reference)
"""Burger dissipative loss operator on 8 TRN2 NeuronCores.

Math (reference):
    u   = x_t[:, 0];  u1 = x_t1[:, 0];  len = edge_attr[:, 0]
    temporal = (u - u1) / dt
    du  = scatter_mean over dst of (u1[dst] - u1[src]) / len
    d2u = scatter_mean over dst of (du[dst] - du[src]) / len
    loss = (temporal + du * u1 - mu * d2u) * mask

Algebraic form used here (per dst d, w = 1/len):
    B[d] = sum_e w[e]*u1[src[e]],  A[d] = sum_e w[e]
    du[d] = (u1[d]*A[d] - B[d]) * inv_c[d],   inv_c = 1/max(deg,1)

Layout: edges partitioned by dst across 8 cores; within a core dsts are
sorted by in-degree ("class") and dealt round-robin onto the 128 SBUF
partitions so that every partition holds the same number of degree-c
dsts (NK[c], padded globally).  A degree-c dst's edges occupy c
consecutive columns, so the per-dst segment sum is a single DVE
tensor_reduce over the innermost axis of a [P, NK[c], c] view — no
scans, no boundary extraction.  Per-edge u1[src]/du[src] values come
from per-column indirect-DMA gathers ([128,1] descriptors per
instruction — the only per-partition-arbitrary gather this hardware
supports).  du is exchanged between rounds with an on-device AllGather.
"""

import os
import sys

for _p in ("/opt/trn_rl_repo", "/root/.axon_site/_ro/trn_rl_repo"):
    if os.path.isdir(_p) and _p not in sys.path:
        sys.path.insert(0, _p)

import numpy as np

import concourse.bass as bass
import concourse.mybir as mybir
import concourse.tile as tile
from concourse import bass_utils
from concourse.vector_clock import ScopedClock

F32 = mybir.dt.float32
I32 = mybir.dt.int32


# --- patch: split the kernel-tail drain's sem waits (walrus rejects CTRL
# instructions with more than a couple of sync waits) -----------------------
_drain_patched = False


def _install_drain_patch():
    global _drain_patched
    if _drain_patched:
        return
    _drain_patched = True

    def _drain_and_barrier(self, tick_clock, wait_clock):
        nc = self.nc
        sink = nc.sync.nop(nofuse=True)
        wait_clock.add_sem_waits(
            sink.ins, ScopedClock({None: tick_clock.global_clock}))
        waits = list(sink.ins.sync_info.on_wait) if sink.ins.sync_info else []
        if len(waits) > 1:
            sink.ins.sync_info = mybir.SyncInfo(
                on_wait=waits[:1], on_update=list(sink.ins.sync_info.on_update))
            rest = waits[1:]
            while rest:
                extra = nc.sync.nop(nofuse=True)
                upd = (list(extra.ins.sync_info.on_update)
                       if extra.ins.sync_info else [])
                extra.ins.sync_info = mybir.SyncInfo(
                    on_wait=rest[:1], on_update=upd)
                rest = rest[1:]
        nc.sync.drain()
        nc.all_engine_barrier()
        assert self.sems is not None
        popped = nc._tile_sem_poison_stack.pop()
        assert popped is self._sem_poison
        nc.clear_and_free_semaphores(list(self.sems.allocated().values()))
        nc.all_engine_barrier()

    tile.TileContext._drain_and_barrier = _drain_and_barrier

    # walrus codegen in this toolchain supports a single sync-wait per
    # instruction; hoist extras onto preceding same-engine NoOps.
    _orig_commit = tile.TileContext._commit_instruction
    _ctr = [0]

    def _commit_instruction(self, inst, lazy_reg_writes=True):
        si = getattr(inst, "sync_info", None)
        if (si is not None and si.on_wait and len(si.on_wait) > 1
                and inst.engine != mybir.EngineType.Unassigned):
            waits = list(si.on_wait)
            inst.sync_info = mybir.SyncInfo(
                on_wait=[waits[-1]], on_update=list(si.on_update))
            for w in waits[:-1]:
                _ctr[0] += 1
                nop = mybir.InstNoOp(name=f"I-ws{_ctr[0]}", ins=[], outs=[])
                nop.engine = inst.engine
                nop.sync_info = mybir.SyncInfo(on_wait=[w], on_update=[])
                self._add_instruction(nop)
        return _orig_commit(self, inst, lazy_reg_writes)

    tile.TileContext._commit_instruction = _commit_instruction


P = 128          # SBUF partitions
NCORES = 8
DELTA_T = 0.01
MU = 0.01


def _strip_dominated_waits(nc, keep_every=256):
    """Thin the semaphore waits carried by the per-column gather DMAs.

    Each indirect gather gets a DMASW ring-lane wait on the gather 8 back
    (the Tile framework's in-flight throttle).  SWDGE generation (~1.09us)
    is slower than DMA completion lag (~1.6us), so at most ~2 gathers are
    ever outstanding and these waits are always satisfied — but processing
    them costs the sequencer ~0.3us per instruction.  Consumers wait on
    final lane totals (one bulk multiply per round), so the waits are
    droppable; keep one in `keep_every` as an in-flight bound for the
    SWDGE descriptor ring.  Also drop any wait dominated by an earlier
    same-engine wait on the same monotone semaphore.
    """
    for fn in nc.m.functions:
        for blk in fn.blocks:
            seen = {}
            gather_ct = 0
            for ins in blk.instructions:
                is_gather = (isinstance(ins, mybir.InstDMACopy)
                             and getattr(ins, "queue", "") == "qPoolDynamic"
                             and ins.engine == mybir.EngineType.Pool)
                si = getattr(ins, "sync_info", None)
                if si is None or not si.on_wait:
                    if is_gather:
                        gather_ct += 1
                    continue
                kept = []
                for w in si.on_wait:
                    if (w.sync_type == "semaphore"
                            and w.wait_mode == "sem-ge-imm"
                            and is_gather and w.ant_name.startswith("DMASW")
                            and gather_ct % keep_every != 0):
                        continue
                    kept.append(w)
                if len(kept) != len(si.on_wait):
                    ins.sync_info = mybir.SyncInfo(
                        on_wait=kept, on_update=list(si.on_update))
                if is_gather:
                    gather_ct += 1


# ---------------------------------------------------------------------------
# Host-side preprocessing: degree-class layout + index construction
# ---------------------------------------------------------------------------

def _preprocess(x_t, x_t1, edge_index, edge_attr, mask, kc):
    N = x_t.shape[0]
    NL = N // NCORES
    assert NL * NCORES == N

    src = np.ascontiguousarray(edge_index[0]).astype(np.int64, copy=False)
    dst = np.ascontiguousarray(edge_index[1]).astype(np.int64, copy=False)
    w_all = (np.float32(1.0) / edge_attr[:, 0].astype(np.float32))

    order = np.argsort(dst, kind="stable")
    ds = dst[order]
    ss = src[order]
    ws = w_all[order]

    # Global degree-class deal: sort ALL nodes by in-degree and deal them
    # round-robin over the 1024 (core, partition) lanes.  Every lane then
    # holds the same number of degree-c nodes (+-1 before padding), which
    # minimises both the per-class padding NK and the edge imbalance.
    deg_all = np.bincount(ds, minlength=N).astype(np.int64)
    KMAX = int(deg_all.max())
    order_nodes_g = np.argsort(deg_all, kind="stable")
    pos_g = np.empty(N, np.int64)
    pos_g[order_nodes_g] = np.arange(N)
    cnt_g = np.bincount(deg_all, minlength=KMAX + 1)
    cstart_g = np.concatenate([[0], np.cumsum(cnt_g)])
    i_g = pos_g - cstart_g[deg_all]          # rank within class, global
    lane_g = i_g % (NCORES * P)              # 0..1023
    core_of = lane_g // P                    # node -> core
    p_of = lane_g % P                        # node -> partition
    s_of = i_g // (NCORES * P)               # slot within class

    NK = -(-cnt_g // (NCORES * P))           # slots per lane per class
    d_off = np.concatenate([[0], np.cumsum(NK)]).astype(np.int64)
    Cb = int(d_off[-1])
    e_off = np.concatenate(
        [[0], np.cumsum(NK * np.arange(KMAX + 1))]).astype(np.int64)
    Craw = int(e_off[-1])
    C = -(-Craw // 4) * 4           # pad to 16B alignment; chunks handle tails
    n_chunks = -(-C // kc)
    DUL = P * Cb

    u1_full = np.ascontiguousarray(x_t1[:, 0]).astype(np.float32)
    u_full = np.ascontiguousarray(x_t[:, 0]).astype(np.float32)
    m_full = np.ascontiguousarray(mask[:, 0]).astype(np.float32)

    slot_all = d_off[deg_all] + s_of              # node -> dst slot
    g_of_node = core_of * DUL + p_of * Cb + slot_all
    inv_all = (1.0 / np.maximum(deg_all, 1)).astype(np.float32)

    # per-edge placement (dst-sorted stream, computed globally)
    E = ds.shape[0]
    cumdeg = np.concatenate([[0], np.cumsum(deg_all)])
    j_e = np.arange(E) - cumdeg[ds]               # edge rank within dst
    c_e = deg_all[ds]
    col_e = e_off[c_e] + s_of[ds] * c_e + j_e
    p_e = p_of[ds]
    core_e = core_of[ds]
    src2_val = g_of_node[ss]

    in_maps = []
    meta = []
    table1 = u1_full.reshape(N, 1)
    for k in range(NCORES):
        em = core_e == k
        src1 = np.zeros((P, C), np.int32)
        wl = np.zeros((P, C), np.float32)
        src2 = np.zeros((P, C), np.int32)
        pe, ce = p_e[em], col_e[em]
        src1[pe, ce] = ss[em]
        wl[pe, ce] = ws[em]
        src2[pe, ce] = src2_val[em]

        gids = np.nonzero(core_of == k)[0]
        pn, sn = p_of[gids], slot_all[gids]
        u1_loc = np.zeros((P, Cb), np.float32)
        u_loc = np.zeros((P, Cb), np.float32)
        m_loc = np.zeros((P, Cb), np.float32)
        inv_c = np.zeros((P, Cb), np.float32)
        u1_loc[pn, sn] = u1_full[gids]
        u_loc[pn, sn] = u_full[gids]
        m_loc[pn, sn] = m_full[gids]
        inv_c[pn, sn] = inv_all[gids]
        perm = np.full((P, Cb), -1, np.int64)
        perm[pn, sn] = gids

        in_maps.append(dict(
            table1=table1, src1=src1, src2=src2, w=wl,
            u1_loc=u1_loc, u_loc=u_loc, m_loc=m_loc, inv_c=inv_c))
        meta.append(perm)
    dims = dict(N=N, NL=NL, C=C, Cb=Cb, DUL=DUL, kc=kc, n_chunks=n_chunks,
                KMAX=KMAX, NK=NK, d_off=d_off, e_off=e_off)
    return in_maps, meta, dims


# ---------------------------------------------------------------------------
# Device kernel
# ---------------------------------------------------------------------------

def _build_nc(dims, ncores=NCORES, nq=1, keep_every=1, use_loop=False):
    N, C, Cb, DUL = dims["N"], dims["C"], dims["Cb"], dims["DUL"]
    kc, n_chunks = dims["kc"], dims["n_chunks"]
    KMAX, NK = dims["KMAX"], dims["NK"]
    d_off, e_off = dims["d_off"], dims["e_off"]
    add = mybir.AluOpType.add
    sub = mybir.AluOpType.subtract
    mult = mybir.AluOpType.mult
    byp = mybir.AluOpType.bypass

    _install_drain_patch()
    nc = bass.Bass("TRN2", target_bir_lowering=False, debug=False,
                   num_devices=ncores, num_swdge_queues=nq)

    def _gather(out_ap, table_ap, off_ap, qi):
        """indirect_dma_start with SWDGE queue selection (round-robin)."""
        eng = nc.gpsimd
        out_l = eng.lower_ap_dma(out_ap, for_indirect_dma=True)
        in_l = eng.lower_ap_dma(table_ap, for_indirect_dma=True)
        off_l = eng.lower_ap_dma(off_ap)
        assert len(in_l) == 1 and len(out_l) == 1 and len(off_l) == 1
        in_l.append(off_l[0])
        ap_shape = table_ap.shape
        coef = 1
        for i in range(1, len(ap_shape)):
            coef *= ap_shape[i]
        in_l[0].dynamic_ap_info = mybir.DynamicAccessPatternInfo(
            c=0,
            actual_ap=out_ap.ap,
            indirect_dim_max_index=ap_shape[0],
            offset_expr=[
                mybir.DynamicAccessPatternOffsetExpr(
                    coef=coef,
                    aff_expr=mybir.DynamicAccessPatternOffsetExprAffExpr(
                        kind="IndirectArgId", arg_id=1,
                    ),
                )
            ],
        )
        return eng.add_instruction(
            mybir.InstDMACopy(
                name=nc.get_next_instruction_name(),
                queue=f"qPoolDynamic{qi or ''}",
                mode="Copy",
                ins=in_l, outs=out_l, oob_is_err=True,
                cce_op=mybir.AluOpType.bypass,
            )
        )

    table1 = nc.dram_tensor("table1", [N, 1], F32, kind="ExternalInput")
    src1_d = nc.dram_tensor("src1", [P, C], I32, kind="ExternalInput")
    src2_d = nc.dram_tensor("src2", [P, C], I32, kind="ExternalInput")
    w_d = nc.dram_tensor("w", [P, C], F32, kind="ExternalInput")
    u1_loc_d = nc.dram_tensor("u1_loc", [P, Cb], F32, kind="ExternalInput")
    u_loc_d = nc.dram_tensor("u_loc", [P, Cb], F32, kind="ExternalInput")
    m_loc_d = nc.dram_tensor("m_loc", [P, Cb], F32, kind="ExternalInput")
    inv_c_d = nc.dram_tensor("inv_c", [P, Cb], F32, kind="ExternalInput")
    loss_d = nc.dram_tensor("loss", [P, Cb], F32, kind="ExternalOutput")

    du_slice = nc.dram_tensor("du_slice", [DUL], F32)
    du_full = nc.dram_tensor("du_full", [ncores * DUL, 1], F32)

    def class_sums(out_t, src_t, classes=None):
        """out[:, slot(c)] = per-dst sums of src_t's class-c edge columns."""
        if classes is None:
            classes = range(0, KMAX + 1)
        for c in classes:
            if c == 0:
                if NK[0] > 0:
                    nc.vector.memset(out_t[:, 0:int(NK[0])], 0.0)
                continue
            nkc = int(NK[c])
            if nkc == 0:
                continue
            a, b = int(e_off[c]), int(e_off[c] + c * nkc)
            dv = slice(int(d_off[c]), int(d_off[c] + nkc))
            view = src_t[:, a:b].rearrange("p (s c) -> p s c", c=c)
            if c == 1:
                nc.vector.tensor_copy(out=out_t[:, dv], in_=src_t[:, a:b])
            else:
                nc.vector.tensor_reduce(out=out_t[:, dv], in_=view,
                                        axis=mybir.AxisListType.X, op=add)

    def classes_done_by(col_end, emitted):
        """Classes whose edge columns all lie before col_end, not yet emitted."""
        out = []
        for c in range(1, KMAX + 1):
            if c in emitted or NK[c] == 0:
                continue
            if int(e_off[c] + c * NK[c]) <= col_end:
                out.append(c)
                emitted.add(c)
        return out

    with tile.TileContext(nc) as tc:
        with tc.tile_pool(name="persist", bufs=1) as pp, \
             tc.tile_pool(name="stream", bufs=3) as sp:

            W_t = pp.tile([P, C], F32, tag="W")
            nc.sync.dma_start(out=W_t[:], in_=w_d[:])
            G_t = pp.tile([P, C], F32, tag="G")
            u1_t = pp.tile([P, Cb], F32, tag="u1")
            nc.sync.dma_start(out=u1_t[:], in_=u1_loc_d[:])
            invc_t = pp.tile([P, Cb], F32, tag="invc")
            nc.sync.dma_start(out=invc_t[:], in_=inv_c_d[:])
            A_t = pp.tile([P, Cb], F32, tag="A")
            B_t = pp.tile([P, Cb], F32, tag="B")
            du_t = pp.tile([P, Cb], F32, tag="du")
            tmp_t = pp.tile([P, Cb], F32, tag="tmp")

            # ---- round 1: gather u1[src] per column, weight, class sums ----
            if use_loop:
                idxp_t = pp.tile([P, C], I32, tag="IDXP")
                nc.sync.dma_start(out=idxp_t[:], in_=src1_d[:])
                with tc.For_i(0, C, name="g1") as li:
                    nc.gpsimd.indirect_dma_start(
                        out=G_t[:, bass.ds(li, 1)], out_offset=None,
                        in_=table1[:],
                        in_offset=bass.IndirectOffsetOnAxis(
                            ap=idxp_t[:, bass.ds(li, 1)], axis=0))
                nc.vector.tensor_tensor(out=G_t[:], in0=G_t[:],
                                        in1=W_t[:], op=mult)
                class_sums(B_t, G_t)
                class_sums(A_t, W_t)
            else:
                # A-sums depend only on W: emit before the gathers (DVE idle)
                class_sums(A_t, W_t)
                class_sums(B_t, G_t, classes=[0])
                emitted = set()
                for j0 in range(0, C, kc):
                    kcj = min(kc, C - j0)
                    cs = slice(j0, j0 + kcj)
                    idx_t = sp.tile([P, kc], I32, tag="idx")
                    nc.sync.dma_start(out=idx_t[:, :kcj], in_=src1_d[:, cs])
                    for i in range(kcj):
                        col = j0 + i
                        _gather(G_t[:, col:col + 1], table1[:],
                                idx_t[:, i:i + 1], col % nq)
                    nc.vector.tensor_tensor(out=G_t[:, cs], in0=G_t[:, cs],
                                            in1=W_t[:, cs], op=mult)
                    class_sums(B_t, G_t,
                               classes=classes_done_by(j0 + kcj, emitted))

            # du = (u1*A - B) * inv_c
            nc.vector.tensor_tensor(out=tmp_t[:], in0=u1_t[:], in1=A_t[:],
                                    op=mult)
            nc.vector.tensor_tensor(out=tmp_t[:], in0=tmp_t[:], in1=B_t[:],
                                    op=sub)
            nc.vector.tensor_tensor(out=du_t[:], in0=tmp_t[:], in1=invc_t[:],
                                    op=mult)

            # ---- allgather du ----------------------------------------------
            nc.sync.dma_start(
                out=du_slice[:].rearrange("(p c) -> p c", p=P), in_=du_t[:])
            nc.gpsimd.collective_compute(
                "AllGather", byp, replica_groups=[list(range(ncores))],
                ins=[du_slice.ap().opt()],
                outs=[du_full.ap().rearrange("n one -> (n one)").opt()])

            # ---- round 2: gather du[src], weight, class sums ---------------
            if use_loop:
                nc.sync.dma_start(out=idxp_t[:], in_=src2_d[:])
                with tc.For_i(0, C, name="g2") as li:
                    nc.gpsimd.indirect_dma_start(
                        out=G_t[:, bass.ds(li, 1)], out_offset=None,
                        in_=du_full[:],
                        in_offset=bass.IndirectOffsetOnAxis(
                            ap=idxp_t[:, bass.ds(li, 1)], axis=0))
                nc.vector.tensor_tensor(out=G_t[:], in0=G_t[:],
                                        in1=W_t[:], op=mult)
            else:
                # u/m for the final loss: load during round-2 gathers
                u_t = pp.tile([P, Cb], F32, tag="u2")
                nc.sync.dma_start(out=u_t[:], in_=u_loc_d[:])
                m_t = pp.tile([P, Cb], F32, tag="m2")
                nc.sync.dma_start(out=m_t[:], in_=m_loc_d[:])
                emitted2 = set()
                for j0 in range(0, C, kc):
                    kcj = min(kc, C - j0)
                    cs = slice(j0, j0 + kcj)
                    idx_t = sp.tile([P, kc], I32, tag="idx")
                    nc.sync.dma_start(out=idx_t[:, :kcj], in_=src2_d[:, cs])
                    for i in range(kcj):
                        col = j0 + i
                        _gather(G_t[:, col:col + 1], du_full[:],
                                idx_t[:, i:i + 1], col % nq)
                    nc.vector.tensor_tensor(out=G_t[:, cs], in0=G_t[:, cs],
                                            in1=W_t[:, cs], op=mult)
                    class_sums(B_t, G_t,
                               classes=classes_done_by(j0 + kcj, emitted2))
            if use_loop:
                class_sums(B_t, G_t)

            # d2u = (du*A - B2) * inv_c   -> tmp_t
            nc.vector.tensor_tensor(out=tmp_t[:], in0=du_t[:], in1=A_t[:],
                                    op=mult)
            nc.vector.tensor_tensor(out=tmp_t[:], in0=tmp_t[:], in1=B_t[:],
                                    op=sub)
            nc.vector.tensor_tensor(out=tmp_t[:], in0=tmp_t[:], in1=invc_t[:],
                                    op=mult)

            # ---- final loss ------------------------------------------------
            if use_loop:
                u_t = pp.tile([P, Cb], F32, tag="u2")
                nc.sync.dma_start(out=u_t[:], in_=u_loc_d[:])
                m_t = pp.tile([P, Cb], F32, tag="m2")
                nc.sync.dma_start(out=m_t[:], in_=m_loc_d[:])
            # du := du*u1 (b-term); u1 dead after
            nc.vector.tensor_tensor(out=du_t[:], in0=du_t[:], in1=u1_t[:],
                                    op=mult)
            # u1 := u - u1
            nc.vector.tensor_tensor(out=u1_t[:], in0=u_t[:], in1=u1_t[:],
                                    op=sub)
            # u1 = u1/dt + du*u1
            nc.vector.scalar_tensor_tensor(
                out=u1_t[:], in0=u1_t[:], scalar=1.0 / DELTA_T, in1=du_t[:],
                op0=mult, op1=add)
            # u1 = -mu*d2u + u1
            nc.vector.scalar_tensor_tensor(
                out=u1_t[:], in0=tmp_t[:], scalar=-MU, in1=u1_t[:],
                op0=mult, op1=add)
            nc.vector.tensor_tensor(out=invc_t[:], in0=u1_t[:], in1=m_t[:],
                                    op=mult)
            nc.sync.dma_start(out=loss_d[:], in_=invc_t[:])

    if keep_every > 0:
        _strip_dominated_waits(nc, keep_every=keep_every)
    return nc


# ---------------------------------------------------------------------------
# Entry point
# ---------------------------------------------------------------------------

def kernel(x_t, x_t1, edge_index, edge_attr, mask, _kc=256, _nq=1,
           _keep=1, _loop=False, _trace=False):
    x_t = np.asarray(x_t)
    x_t1 = np.asarray(x_t1)
    edge_index = np.asarray(edge_index)
    edge_attr = np.asarray(edge_attr)
    mask = np.asarray(mask)
    N = x_t.shape[0]

    in_maps, meta, dims = _preprocess(x_t, x_t1, edge_index, edge_attr, mask,
                                      _kc)
    nc = _build_nc(dims, nq=_nq, keep_every=_keep, use_loop=_loop)
    res = bass_utils.run_bass_kernel_spmd(
        nc, in_maps, core_ids=list(range(NCORES)), trace=_trace)

    out = np.empty(N, np.float32)
    for k in range(NCORES):
        loss_k = res.results[k]["loss"].reshape(-1)
        perm_k = meta[k].reshape(-1)
        valid = perm_k >= 0
        out[perm_k[valid]] = loss_k[valid]
    if _trace:
        kernel._last_results = res
    return out


# revision 34
# speedup vs baseline: 1.0001x; 1.0001x over previous
"""Burger dissipative loss operator on 8 TRN2 NeuronCores.

Math (reference):
    u   = x_t[:, 0];  u1 = x_t1[:, 0];  len = edge_attr[:, 0]
    temporal = (u - u1) / dt
    du  = scatter_mean over dst of (u1[dst] - u1[src]) / len
    d2u = scatter_mean over dst of (du[dst] - du[src]) / len
    loss = (temporal + du * u1 - mu * d2u) * mask

Algebraic form used here (per dst d, w = 1/len):
    B[d] = sum_e w[e]*u1[src[e]],  A[d] = sum_e w[e]
    du[d] = (u1[d]*A[d] - B[d]) * inv_c[d],   inv_c = 1/max(deg,1)

Layout: edges partitioned by dst across 8 cores; within a core dsts are
sorted by in-degree ("class") and dealt round-robin onto the 128 SBUF
partitions so that every partition holds the same number of degree-c
dsts (NK[c], padded globally).  A degree-c dst's edges occupy c
consecutive columns, so the per-dst segment sum is a single DVE
tensor_reduce over the innermost axis of a [P, NK[c], c] view — no
scans, no boundary extraction.  Per-edge u1[src]/du[src] values come
from per-column indirect-DMA gathers ([128,1] descriptors per
instruction — the only per-partition-arbitrary gather this hardware
supports).  du is exchanged between rounds with an on-device AllGather.
"""

import os
import sys

for _p in ("/opt/trn_rl_repo", "/root/.axon_site/_ro/trn_rl_repo"):
    if os.path.isdir(_p) and _p not in sys.path:
        sys.path.insert(0, _p)

import numpy as np

import concourse.bass as bass
import concourse.mybir as mybir
import concourse.tile as tile
from concourse import bass_utils
from concourse.vector_clock import ScopedClock

F32 = mybir.dt.float32
I32 = mybir.dt.int32


# --- patch: split the kernel-tail drain's sem waits (walrus rejects CTRL
# instructions with more than a couple of sync waits) -----------------------
_drain_patched = False


def _install_drain_patch():
    global _drain_patched
    if _drain_patched:
        return
    _drain_patched = True

    def _drain_and_barrier(self, tick_clock, wait_clock):
        nc = self.nc
        sink = nc.sync.nop(nofuse=True)
        wait_clock.add_sem_waits(
            sink.ins, ScopedClock({None: tick_clock.global_clock}))
        waits = list(sink.ins.sync_info.on_wait) if sink.ins.sync_info else []
        if len(waits) > 1:
            sink.ins.sync_info = mybir.SyncInfo(
                on_wait=waits[:1], on_update=list(sink.ins.sync_info.on_update))
            rest = waits[1:]
            while rest:
                extra = nc.sync.nop(nofuse=True)
                upd = (list(extra.ins.sync_info.on_update)
                       if extra.ins.sync_info else [])
                extra.ins.sync_info = mybir.SyncInfo(
                    on_wait=rest[:1], on_update=upd)
                rest = rest[1:]
        nc.sync.drain()
        nc.all_engine_barrier()
        assert self.sems is not None
        popped = nc._tile_sem_poison_stack.pop()
        assert popped is self._sem_poison
        nc.clear_and_free_semaphores(list(self.sems.allocated().values()))
        nc.all_engine_barrier()

    tile.TileContext._drain_and_barrier = _drain_and_barrier

    # walrus codegen in this toolchain supports a single sync-wait per
    # instruction; hoist extras onto preceding same-engine NoOps.
    _orig_commit = tile.TileContext._commit_instruction
    _ctr = [0]

    def _commit_instruction(self, inst, lazy_reg_writes=True):
        si = getattr(inst, "sync_info", None)
        if (si is not None and si.on_wait and len(si.on_wait) > 1
                and inst.engine != mybir.EngineType.Unassigned):
            waits = list(si.on_wait)
            inst.sync_info = mybir.SyncInfo(
                on_wait=[waits[-1]], on_update=list(si.on_update))
            for w in waits[:-1]:
                _ctr[0] += 1
                nop = mybir.InstNoOp(name=f"I-ws{_ctr[0]}", ins=[], outs=[])
                nop.engine = inst.engine
                nop.sync_info = mybir.SyncInfo(on_wait=[w], on_update=[])
                self._add_instruction(nop)
        return _orig_commit(self, inst, lazy_reg_writes)

    tile.TileContext._commit_instruction = _commit_instruction


P = 128          # SBUF partitions
NCORES = 8
DELTA_T = 0.01
MU = 0.01


def _strip_dominated_waits(nc, keep_every=256):
    """Thin the semaphore waits carried by the per-column gather DMAs.

    Each indirect gather gets a DMASW ring-lane wait on the gather 8 back
    (the Tile framework's in-flight throttle).  SWDGE generation (~1.09us)
    is slower than DMA completion lag (~1.6us), so at most ~2 gathers are
    ever outstanding and these waits are always satisfied — but processing
    them costs the sequencer ~0.3us per instruction.  Consumers wait on
    final lane totals (one bulk multiply per round), so the waits are
    droppable; keep one in `keep_every` as an in-flight bound for the
    SWDGE descriptor ring.  Also drop any wait dominated by an earlier
    same-engine wait on the same monotone semaphore.
    """
    for fn in nc.m.functions:
        for blk in fn.blocks:
            seen = {}
            gather_ct = 0
            for ins in blk.instructions:
                is_gather = (isinstance(ins, mybir.InstDMACopy)
                             and getattr(ins, "queue", "") == "qPoolDynamic"
                             and ins.engine == mybir.EngineType.Pool)
                si = getattr(ins, "sync_info", None)
                if si is None or not si.on_wait:
                    if is_gather:
                        gather_ct += 1
                    continue
                kept = []
                for w in si.on_wait:
                    if (w.sync_type == "semaphore"
                            and w.wait_mode == "sem-ge-imm"
                            and is_gather and w.ant_name.startswith("DMASW")
                            and gather_ct % keep_every != 0):
                        continue
                    kept.append(w)
                if len(kept) != len(si.on_wait):
                    ins.sync_info = mybir.SyncInfo(
                        on_wait=kept, on_update=list(si.on_update))
                if is_gather:
                    gather_ct += 1


# ---------------------------------------------------------------------------
# Host-side preprocessing: degree-class layout + index construction
# ---------------------------------------------------------------------------

def _preprocess(x_t, x_t1, edge_index, edge_attr, mask, kc):
    N = x_t.shape[0]
    NL = N // NCORES
    assert NL * NCORES == N

    src = np.ascontiguousarray(edge_index[0]).astype(np.int64, copy=False)
    dst = np.ascontiguousarray(edge_index[1]).astype(np.int64, copy=False)
    w_all = (np.float32(1.0) / edge_attr[:, 0].astype(np.float32))

    order = np.argsort(dst, kind="stable")
    ds = dst[order]
    ss = src[order]
    ws = w_all[order]

    # Global degree-class deal: sort ALL nodes by in-degree and deal them
    # round-robin over the 1024 (core, partition) lanes.  Every lane then
    # holds the same number of degree-c nodes (+-1 before padding), which
    # minimises both the per-class padding NK and the edge imbalance.
    deg_all = np.bincount(ds, minlength=N).astype(np.int64)
    KMAX = int(deg_all.max())
    order_nodes_g = np.argsort(deg_all, kind="stable")
    pos_g = np.empty(N, np.int64)
    pos_g[order_nodes_g] = np.arange(N)
    cnt_g = np.bincount(deg_all, minlength=KMAX + 1)
    cstart_g = np.concatenate([[0], np.cumsum(cnt_g)])
    i_g = pos_g - cstart_g[deg_all]          # rank within class, global
    lane_g = i_g % (NCORES * P)              # 0..1023
    core_of = lane_g // P                    # node -> core
    p_of = lane_g % P                        # node -> partition
    s_of = i_g // (NCORES * P)               # slot within class

    NK = -(-cnt_g // (NCORES * P))           # slots per lane per class
    d_off = np.concatenate([[0], np.cumsum(NK)]).astype(np.int64)
    Cb = int(d_off[-1])
    e_off = np.concatenate(
        [[0], np.cumsum(NK * np.arange(KMAX + 1))]).astype(np.int64)
    Craw = int(e_off[-1])
    C = -(-Craw // 4) * 4           # pad to 16B alignment; chunks handle tails
    n_chunks = -(-C // kc)
    DUL = P * Cb

    u1_full = np.ascontiguousarray(x_t1[:, 0]).astype(np.float32)
    u_full = np.ascontiguousarray(x_t[:, 0]).astype(np.float32)
    m_full = np.ascontiguousarray(mask[:, 0]).astype(np.float32)

    slot_all = d_off[deg_all] + s_of              # node -> dst slot
    g_of_node = core_of * DUL + p_of * Cb + slot_all
    inv_all = (1.0 / np.maximum(deg_all, 1)).astype(np.float32)

    # per-edge placement (dst-sorted stream, computed globally)
    E = ds.shape[0]
    cumdeg = np.concatenate([[0], np.cumsum(deg_all)])
    j_e = np.arange(E) - cumdeg[ds]               # edge rank within dst
    c_e = deg_all[ds]
    col_e = e_off[c_e] + s_of[ds] * c_e + j_e
    p_e = p_of[ds]
    core_e = core_of[ds]
    src2_val = g_of_node[ss]

    in_maps = []
    meta = []
    table1 = u1_full.reshape(N, 1)
    for k in range(NCORES):
        em = core_e == k
        src1 = np.zeros((P, C), np.int32)
        wl = np.zeros((P, C), np.float32)
        src2 = np.zeros((P, C), np.int32)
        pe, ce = p_e[em], col_e[em]
        src1[pe, ce] = ss[em]
        wl[pe, ce] = ws[em]
        src2[pe, ce] = src2_val[em]

        gids = np.nonzero(core_of == k)[0]
        pn, sn = p_of[gids], slot_all[gids]
        u1_loc = np.zeros((P, Cb), np.float32)
        u_loc = np.zeros((P, Cb), np.float32)
        m_loc = np.zeros((P, Cb), np.float32)
        inv_c = np.zeros((P, Cb), np.float32)
        u1_loc[pn, sn] = u1_full[gids]
        u_loc[pn, sn] = u_full[gids]
        m_loc[pn, sn] = m_full[gids]
        inv_c[pn, sn] = inv_all[gids]
        perm = np.full((P, Cb), -1, np.int64)
        perm[pn, sn] = gids

        in_maps.append(dict(
            table1=table1, src1=src1, src2=src2, w=wl,
            u1_loc=u1_loc, u_loc=u_loc, m_loc=m_loc, inv_c=inv_c))
        meta.append(perm)
    dims = dict(N=N, NL=NL, C=C, Cb=Cb, DUL=DUL, kc=kc, n_chunks=n_chunks,
                KMAX=KMAX, NK=NK, d_off=d_off, e_off=e_off)
    return in_maps, meta, dims


# ---------------------------------------------------------------------------
# Device kernel
# ---------------------------------------------------------------------------

def _build_nc(dims, ncores=NCORES, nq=1, keep_every=1, use_loop=False):
    N, C, Cb, DUL = dims["N"], dims["C"], dims["Cb"], dims["DUL"]
    kc, n_chunks = dims["kc"], dims["n_chunks"]
    KMAX, NK = dims["KMAX"], dims["NK"]
    d_off, e_off = dims["d_off"], dims["e_off"]
    add = mybir.AluOpType.add
    sub = mybir.AluOpType.subtract
    mult = mybir.AluOpType.mult
    byp = mybir.AluOpType.bypass

    _install_drain_patch()
    nc = bass.Bass("TRN2", target_bir_lowering=False, debug=False,
                   num_devices=ncores, num_swdge_queues=nq)

    def _gather(out_ap, table_ap, off_ap, qi):
        """indirect_dma_start with SWDGE queue selection (round-robin)."""
        eng = nc.gpsimd
        out_l = eng.lower_ap_dma(out_ap, for_indirect_dma=True)
        in_l = eng.lower_ap_dma(table_ap, for_indirect_dma=True)
        off_l = eng.lower_ap_dma(off_ap)
        assert len(in_l) == 1 and len(out_l) == 1 and len(off_l) == 1
        in_l.append(off_l[0])
        ap_shape = table_ap.shape
        coef = 1
        for i in range(1, len(ap_shape)):
            coef *= ap_shape[i]
        in_l[0].dynamic_ap_info = mybir.DynamicAccessPatternInfo(
            c=0,
            actual_ap=out_ap.ap,
            indirect_dim_max_index=ap_shape[0],
            offset_expr=[
                mybir.DynamicAccessPatternOffsetExpr(
                    coef=coef,
                    aff_expr=mybir.DynamicAccessPatternOffsetExprAffExpr(
                        kind="IndirectArgId", arg_id=1,
                    ),
                )
            ],
        )
        return eng.add_instruction(
            mybir.InstDMACopy(
                name=nc.get_next_instruction_name(),
                queue=f"qPoolDynamic{qi or ''}",
                mode="Copy",
                ins=in_l, outs=out_l, oob_is_err=True,
                cce_op=mybir.AluOpType.bypass,
            )
        )

    table1 = nc.dram_tensor("table1", [N, 1], F32, kind="ExternalInput")
    src1_d = nc.dram_tensor("src1", [P, C], I32, kind="ExternalInput")
    src2_d = nc.dram_tensor("src2", [P, C], I32, kind="ExternalInput")
    w_d = nc.dram_tensor("w", [P, C], F32, kind="ExternalInput")
    u1_loc_d = nc.dram_tensor("u1_loc", [P, Cb], F32, kind="ExternalInput")
    u_loc_d = nc.dram_tensor("u_loc", [P, Cb], F32, kind="ExternalInput")
    m_loc_d = nc.dram_tensor("m_loc", [P, Cb], F32, kind="ExternalInput")
    inv_c_d = nc.dram_tensor("inv_c", [P, Cb], F32, kind="ExternalInput")
    loss_d = nc.dram_tensor("loss", [P, Cb], F32, kind="ExternalOutput")

    du_slice = nc.dram_tensor("du_slice", [DUL], F32)
    du_full = nc.dram_tensor("du_full", [ncores * DUL, 1], F32)

    def class_sums(out_t, src_t, classes=None):
        """out[:, slot(c)] = per-dst sums of src_t's class-c edge columns."""
        if classes is None:
            classes = range(0, KMAX + 1)
        for c in classes:
            if c == 0:
                if NK[0] > 0:
                    nc.vector.memset(out_t[:, 0:int(NK[0])], 0.0)
                continue
            nkc = int(NK[c])
            if nkc == 0:
                continue
            a, b = int(e_off[c]), int(e_off[c] + c * nkc)
            dv = slice(int(d_off[c]), int(d_off[c] + nkc))
            view = src_t[:, a:b].rearrange("p (s c) -> p s c", c=c)
            if c == 1:
                nc.vector.tensor_copy(out=out_t[:, dv], in_=src_t[:, a:b])
            else:
                nc.vector.tensor_reduce(out=out_t[:, dv], in_=view,
                                        axis=mybir.AxisListType.X, op=add)

    def classes_done_by(col_end, emitted):
        """Classes whose edge columns all lie before col_end, not yet emitted."""
        out = []
        for c in range(1, KMAX + 1):
            if c in emitted or NK[c] == 0:
                continue
            if int(e_off[c] + c * NK[c]) <= col_end:
                out.append(c)
                emitted.add(c)
        return out

    with tile.TileContext(nc) as tc:
        with tc.tile_pool(name="persist", bufs=1) as pp, \
             tc.tile_pool(name="stream", bufs=3) as sp:

            W_t = pp.tile([P, C], F32, tag="W")
            nc.sync.dma_start(out=W_t[:], in_=w_d[:])
            G_t = pp.tile([P, C], F32, tag="G")
            u1_t = pp.tile([P, Cb], F32, tag="u1")
            nc.sync.dma_start(out=u1_t[:], in_=u1_loc_d[:])
            invc_t = pp.tile([P, Cb], F32, tag="invc")
            nc.sync.dma_start(out=invc_t[:], in_=inv_c_d[:])
            A_t = pp.tile([P, Cb], F32, tag="A")
            B_t = pp.tile([P, Cb], F32, tag="B")
            du_t = pp.tile([P, Cb], F32, tag="du")
            tmp_t = pp.tile([P, Cb], F32, tag="tmp")

            # ---- round 1: gather u1[src] per column, weight, class sums ----
            if use_loop:
                idxp_t = pp.tile([P, C], I32, tag="IDXP")
                nc.sync.dma_start(out=idxp_t[:], in_=src1_d[:])
                with tc.For_i(0, C, name="g1") as li:
                    nc.gpsimd.indirect_dma_start(
                        out=G_t[:, bass.ds(li, 1)], out_offset=None,
                        in_=table1[:],
                        in_offset=bass.IndirectOffsetOnAxis(
                            ap=idxp_t[:, bass.ds(li, 1)], axis=0))
                nc.vector.tensor_tensor(out=G_t[:], in0=G_t[:],
                                        in1=W_t[:], op=mult)
            else:
                for j0 in range(0, C, kc):
                    kcj = min(kc, C - j0)
                    cs = slice(j0, j0 + kcj)
                    idx_t = sp.tile([P, kc], I32, tag="idx")
                    nc.sync.dma_start(out=idx_t[:, :kcj], in_=src1_d[:, cs])
                    for i in range(kcj):
                        col = j0 + i
                        _gather(G_t[:, col:col + 1], table1[:],
                                idx_t[:, i:i + 1], col % nq)
                    nc.vector.tensor_tensor(out=G_t[:, cs], in0=G_t[:, cs],
                                            in1=W_t[:, cs], op=mult)
            class_sums(B_t, G_t)
            class_sums(A_t, W_t)

            # du = (u1*A - B) * inv_c
            nc.vector.tensor_tensor(out=tmp_t[:], in0=u1_t[:], in1=A_t[:],
                                    op=mult)
            nc.vector.tensor_tensor(out=tmp_t[:], in0=tmp_t[:], in1=B_t[:],
                                    op=sub)
            nc.vector.tensor_tensor(out=du_t[:], in0=tmp_t[:], in1=invc_t[:],
                                    op=mult)

            # ---- allgather du ----------------------------------------------
            nc.sync.dma_start(
                out=du_slice[:].rearrange("(p c) -> p c", p=P), in_=du_t[:])
            nc.gpsimd.collective_compute(
                "AllGather", byp, replica_groups=[list(range(ncores))],
                ins=[du_slice.ap().opt()],
                outs=[du_full.ap().rearrange("n one -> (n one)").opt()])

            # ---- round 2: gather du[src], weight, class sums ---------------
            if use_loop:
                nc.sync.dma_start(out=idxp_t[:], in_=src2_d[:])
                with tc.For_i(0, C, name="g2") as li:
                    nc.gpsimd.indirect_dma_start(
                        out=G_t[:, bass.ds(li, 1)], out_offset=None,
                        in_=du_full[:],
                        in_offset=bass.IndirectOffsetOnAxis(
                            ap=idxp_t[:, bass.ds(li, 1)], axis=0))
                nc.vector.tensor_tensor(out=G_t[:], in0=G_t[:],
                                        in1=W_t[:], op=mult)
            else:
                for j0 in range(0, C, kc):
                    kcj = min(kc, C - j0)
                    cs = slice(j0, j0 + kcj)
                    idx_t = sp.tile([P, kc], I32, tag="idx")
                    nc.sync.dma_start(out=idx_t[:, :kcj], in_=src2_d[:, cs])
                    for i in range(kcj):
                        col = j0 + i
                        _gather(G_t[:, col:col + 1], du_full[:],
                                idx_t[:, i:i + 1], col % nq)
                    nc.vector.tensor_tensor(out=G_t[:, cs], in0=G_t[:, cs],
                                            in1=W_t[:, cs], op=mult)
            class_sums(B_t, G_t)

            # d2u = (du*A - B2) * inv_c   -> tmp_t
            nc.vector.tensor_tensor(out=tmp_t[:], in0=du_t[:], in1=A_t[:],
                                    op=mult)
            nc.vector.tensor_tensor(out=tmp_t[:], in0=tmp_t[:], in1=B_t[:],
                                    op=sub)
            nc.vector.tensor_tensor(out=tmp_t[:], in0=tmp_t[:], in1=invc_t[:],
                                    op=mult)

            # ---- final loss ------------------------------------------------
            u_t = pp.tile([P, Cb], F32, tag="A")       # reuse A slot
            nc.sync.dma_start(out=u_t[:], in_=u_loc_d[:])
            m_t = pp.tile([P, Cb], F32, tag="B")       # reuse B slot
            nc.sync.dma_start(out=m_t[:], in_=m_loc_d[:])
            # du := du*u1 (b-term); u1 dead after
            nc.vector.tensor_tensor(out=du_t[:], in0=du_t[:], in1=u1_t[:],
                                    op=mult)
            # u1 := u - u1
            nc.vector.tensor_tensor(out=u1_t[:], in0=u_t[:], in1=u1_t[:],
                                    op=sub)
            # u1 = u1/dt + du*u1
            nc.vector.scalar_tensor_tensor(
                out=u1_t[:], in0=u1_t[:], scalar=1.0 / DELTA_T, in1=du_t[:],
                op0=mult, op1=add)
            # u1 = -mu*d2u + u1
            nc.vector.scalar_tensor_tensor(
                out=u1_t[:], in0=tmp_t[:], scalar=-MU, in1=u1_t[:],
                op0=mult, op1=add)
            nc.vector.tensor_tensor(out=invc_t[:], in0=u1_t[:], in1=m_t[:],
                                    op=mult)
            nc.sync.dma_start(out=loss_d[:], in_=invc_t[:])

    if keep_every > 0:
        _strip_dominated_waits(nc, keep_every=keep_every)
    return nc


# ---------------------------------------------------------------------------
# Entry point
# ---------------------------------------------------------------------------

def kernel(x_t, x_t1, edge_index, edge_attr, mask, _kc=256, _nq=1,
           _keep=1, _loop=False, _trace=False):
    x_t = np.asarray(x_t)
    x_t1 = np.asarray(x_t1)
    edge_index = np.asarray(edge_index)
    edge_attr = np.asarray(edge_attr)
    mask = np.asarray(mask)
    N = x_t.shape[0]

    in_maps, meta, dims = _preprocess(x_t, x_t1, edge_index, edge_attr, mask,
                                      _kc)
    nc = _build_nc(dims, nq=_nq, keep_every=_keep, use_loop=_loop)
    res = bass_utils.run_bass_kernel_spmd(
        nc, in_maps, core_ids=list(range(NCORES)), trace=_trace)

    out = np.empty(N, np.float32)
    for k in range(NCORES):
        loss_k = res.results[k]["loss"].reshape(-1)
        perm_k = meta[k].reshape(-1)
        valid = perm_k >= 0
        out[perm_k[valid]] = loss_k[valid]
    if _trace:
        kernel._last_results = res
    return out
